# revision 9
# baseline (speedup 1.0000x reference)
"""RGB->hue + 1x1 conv (scalar scale+bias) Trainium2 Bass kernel.

Problem: x [32,3,512,512] f32 -> out [32,1,512,512] f32
  hue6 selected per argmax(r,g,b) branch (first-max, like torch):
    r max: (g-b)/delta  (mod 6)
    g max: (b-r)/delta + 2
    b max: (r-g)/delta + 4
  out = hue6 * (W/6) + b

Sharding: pure data parallel, 4 images per core on 8 cores. Host
re-lays x channel-major per core ([3, P, 8192], images along the free
dim) so compute pieces can be any size: small lead-in/lead-out pieces
hide DMA fill/drain, large middle pieces amortize instruction startup.

Formulation (divide-first, select-late). Let w6 = W/6, s = sign(w6).
With sign-folded diffs dgb' = s*(g-b), dbr' = s*(b-r) (operand swap at
trace time, zero cost):
  delta = max(|dgb'|,|dbr'|,|dgb'+dbr'|)   [custom DVE op ABSMAX3, via
          the sum-zero identity max3 = max(M3, -m3)]
  u  = |w6|/delta  (ACT Reciprocal, input scale 1/|w6|)
  q1w = dgb'*u + 6w6*[dgb'*u wraps]        [custom MULT_WRAP: the
        r-branch mod-6, folded into the multiply]
  q2 = dbr'*u
  m  = q1w>3w6 ? 10w6-(q1w+q2) : q2+2w6    [custom GB_WRAPPED: the g/b
        merge; 10w6 absorbs the +6w6 carried by wrapped q1w, so g/b
        outputs land in [w6,5w6] = w6*hue6 directly, no post-wrap]
  copy_predicated m <- q1w where r-max     [mask from ACT: r-max <=>
        s*(dbr + relu(dgb)) < 0, via max(dbr,dgb+dbr) = dbr+relu(dgb)]
  out = Copy(m) + bias (ACT, writes the fp16 output tile)

All intermediates fp16 (2x DVE modes; 2^-11 rel). Input is downcast to
fp16 on the host before sharding: halves input HBM traffic and makes
the first-touch subtracts 2x; delta can round to 0 for near-achromatic
pixels so the reciprocal gets a small eps bias (sim rel err 3.7e-3 vs
2e-2 budget). INPUT_F32 = True falls back to f32 input planes (exact
diffs, sim rel 2.7e-4) at 2x input DMA and 1x-subtract cost. Output is
fp16, upcast on host.
"""

import numpy as np

_EXE_CACHE: dict = {}

# Layout constants (x [32,3,512,512], 8 cores)
N_CORES = 8
IMGS_PER_CORE = 4
P = 128               # SBUF partitions
PLANE = 512 * 512     # elements per channel plane
FREE = PLANE // P     # 2048 free-dim elements per plane
FTOT = IMGS_PER_CORE * FREE  # 8192 free-dim elements per channel per core
# graded piece sizes: small ends for pipeline fill/drain, big middle
PIECES = [512, 1280, 2560, 2304, 1024, 512]
assert sum(PIECES) == FTOT

INPUT_F32 = False     # False: host downcasts x to fp16 (half the input DMA)

_OPS_CACHE: dict = {}


def _register_custom_ops():
    """Register the fused DVE ops in concourse.dve_ops' runtime tables.
    Name->opcode rows are assigned past the stock OPS list; idempotent."""
    if _OPS_CACHE:
        return _OPS_CACHE
    import concourse.dve_ops as dops
    from concourse.dve_spec import (Spec, Src0, Src1, C0, C1, C2, Zero, maxx,
                                    minn, select, lower, _has_src1)
    from concourse.dve_uop import DveOpSpec

    existing = {op.name: op for op in dops.OPS}

    def make(name, spec):
        if name in existing:
            return existing[name]
        row = dops._CUSTOM_DVE_ROW_BASE + len(dops.OPS)
        shas = {}
        for ver in ("v3",):
            s = DveOpSpec(name=name, opcode=row, uops=lower(spec, ver=ver),
                          rd1_en=_has_src1(spec))
            shas[ver] = s.sha(ver)
        op = dops.DveOp(name, spec, subdim=False, uops_sha=shas)
        dops.OPS.append(op)
        dops.CUSTOM_DVE_SPECS[name] = spec
        dops._SUB_OPCODE_FOR_NAME[name] = row
        return op

    # delta = max(|a|,|b|,|a+b|): {a,b,-(a+b)} sums to zero, so the largest
    # magnitude is max(max3, -min3).
    t1 = Src0 + Src1
    c = Zero - t1
    M3 = maxx(maxx(Src0, Src1), c)
    m3 = minn(minn(Src0, Src1), c)
    spec_absmax3 = Spec(
        body=maxx(M3, Zero - m3),
        reference=lambda in0, in1, s0, s1, imm2: np.maximum(
            np.maximum(np.abs(in0.astype(np.float32)), np.abs(in1)),
            np.abs(in0.astype(np.float32) + in1)).astype(np.float32),
    )
    # q1w = p + C0*[p<0] + C1 (pos) / p + C0*[p>0] + C1 (neg), p = in0*in1,
    # C0 = 6w6, C1 = bias: the r-branch wrap AND the conv bias, fused into
    # the quotient multiply.
    p = Src0 * Src1
    spec_mw_pos = Spec(
        body=p + ((p < Zero) * C0) + C1,
        reference=lambda in0, in1, s0, s1, imm2: (
            in0.astype(np.float32) * in1
            + (in0.astype(np.float32) * in1 < 0) * s0 + s1).astype(np.float32),
    )
    spec_mw_neg = Spec(
        body=p + ((p > Zero) * C0) + C1,
        reference=lambda in0, in1, s0, s1, imm2: (
            in0.astype(np.float32) * in1
            + (in0.astype(np.float32) * in1 > 0) * s0 + s1).astype(np.float32),
    )
    # m = sel(b-branch, C1-(q1w+q2), q2+C0): C0=2w6, C1=10w6, C2(imm2)=3w6.
    # b-branch <=> q1w beyond 3w6 (wrapped q1w sits in [5w6,6w6]).
    tt = Src0 + Src1
    a = Src1 + C0
    b2 = C1 - tt
    spec_gbw_pos = Spec(
        body=select(Src0 > C2, b2, a),
        reference=lambda in0, in1, s0, s1, imm2: np.where(
            in0 > imm2, s1 - (in0.astype(np.float32) + in1),
            in1 + s0).astype(np.float32),
    )
    spec_gbw_neg = Spec(
        body=select(Src0 < C2, b2, a),
        reference=lambda in0, in1, s0, s1, imm2: np.where(
            in0 < imm2, s1 - (in0.astype(np.float32) + in1),
            in1 + s0).astype(np.float32),
    )
    _OPS_CACHE["absmax3"] = make("HUE_ABSMAX3_ANT", spec_absmax3)
    _OPS_CACHE["mw_pos"] = make("HUE_MW_POS_ANT", spec_mw_pos)
    _OPS_CACHE["mw_neg"] = make("HUE_MW_NEG_ANT", spec_mw_neg)
    _OPS_CACHE["gbw_pos"] = make("HUE_GBW_POS_ANT", spec_gbw_pos)
    _OPS_CACHE["gbw_neg"] = make("HUE_GBW_NEG_ANT", spec_gbw_neg)
    return _OPS_CACHE


def _build(w6: float, bias: float, input_f32: bool):
    """Trace the Bass kernel with W/6 and bias baked as immediates."""
    import concourse.bacc as bacc
    import concourse.tile as tile
    from concourse import mybir

    F16 = mybir.dt.float16
    F32 = mybir.dt.float32
    U8 = mybir.dt.uint8
    Alu = mybir.AluOpType
    Act = mybir.ActivationFunctionType

    ops = _register_custom_ops()
    pos = w6 >= 0
    aw6 = abs(w6)
    IN_DT = F32 if input_f32 else F16
    # fp16 input rounding can collapse delta to 0; keep 1/(delta/|w6|+eps)
    # finite there (q's are then 0 too, so the pixel lands on hue 2, bounded).
    recip_eps = 0.0 if input_f32 else 1e-4
    FMAX = max(PIECES)

    nc = bacc.Bacc("TRN2", target_bir_lowering=False, debug=False)

    def act_recip(out_ap, in_ap, scale, bias_f):
        ins = [
            nc.scalar.lower_ap(in_ap),
            mybir.ImmediateValue(dtype=F32, value=bias_f),
            mybir.ImmediateValue(dtype=F32, value=scale),
            mybir.ImmediateValue(dtype=F32, value=0.0),
        ]
        return nc.scalar.add_instruction(
            mybir.InstActivation(
                name=nc.get_next_instruction_name(),
                func=Act.Reciprocal,
                ins=ins,
                outs=[nc.scalar.lower_ap(out_ap)],
            )
        )

    x_t = nc.dram_tensor("x", [3, P, FTOT], IN_DT, kind="ExternalInput")
    o_t = nc.dram_tensor("out", [P, FTOT], F16, kind="ExternalOutput")

    with tile.TileContext(nc, pool_alloc_mode="queue") as tc:
        with (
            tc.tile_pool(name="io", bufs=3) as io,
            tc.tile_pool(name="tmp", bufs=2) as tmp,
        ):
            state = {}
            pieces = []
            c0 = 0
            for w in PIECES:
                pieces.append((c0, w))
                c0 += w

            def stage_a(pi):
                c0, w = pieces[pi]
                r = io.tile([P, FMAX], IN_DT, tag="r", name="r")[:, :w]
                g = io.tile([P, FMAX], IN_DT, tag="g", name="g")[:, :w]
                b = io.tile([P, FMAX], IN_DT, tag="b", name="b")[:, :w]
                nc.sync.dma_start(g, x_t[1, :, c0:c0 + w])
                nc.sync.dma_start(b, x_t[2, :, c0:c0 + w])
                nc.sync.dma_start(r, x_t[0, :, c0:c0 + w])

                # sign-folded diffs: dgb' = s*(g-b), dbr' = s*(b-r)
                dgb = tmp.tile([P, FMAX], F16, tag="dgb", name="dgb")[:, :w]
                dbr = tmp.tile([P, FMAX], F16, tag="dbr", name="dbr")[:, :w]
                if pos:
                    nc.vector.tensor_sub(dgb, g, b)
                    nc.vector.tensor_sub(dbr, b, r)
                else:
                    nc.vector.tensor_sub(dgb, b, g)
                    nc.vector.tensor_sub(dbr, r, b)

                delta = tmp.tile([P, FMAX], F16, tag="delta", name="delta")[:, :w]
                nc.vector._custom_dve(ops["absmax3"], out=delta, in0=dgb, in1=dbr)

                # r-max <=> s*(dbr + relu(dgb)) < 0; with primed operands:
                # pos: w = relu(dgb')+dbr' < 0 ; neg: w = dbr'-relu(-dgb') > 0
                re = tmp.tile([P, FMAX], F16, tag="re", name="re")[:, :w]
                nc.scalar.activation(re, dgb, Act.Relu,
                                     scale=1.0 if pos else -1.0)
                wv = tmp.tile([P, FMAX], F16, tag="wv", name="wv")[:, :w]
                if pos:
                    nc.vector.tensor_add(wv, re, dbr)
                else:
                    nc.vector.tensor_sub(wv, dbr, re)
                mask = tmp.tile([P, FMAX], U8, tag="mask", name="mask")[:, :w]
                nc.scalar.activation(mask, wv, Act.Relu,
                                     scale=-1e4 if pos else 1e4)

                u = tmp.tile([P, FMAX], F16, tag="u", name="u")[:, :w]
                act_recip(u, delta, scale=1.0 / aw6, bias_f=recip_eps)

                state[pi] = (dgb, dbr, u, mask)

            def stage_b(pi):
                c0, w = pieces[pi]
                dgb, dbr, u, mask = state.pop(pi)

                q1w = tmp.tile([P, FMAX], F16, tag="q1w", name="q1w")[:, :w]
                nc.vector._custom_dve(
                    ops["mw_pos" if pos else "mw_neg"], out=q1w, in0=dgb,
                    in1=u, s0=6 * w6, s1=bias)
                q2 = tmp.tile([P, FMAX], F16, tag="q2", name="q2")[:, :w]
                nc.vector.tensor_tensor(q2, dbr, u, op=Alu.mult)

                # bias is folded into every branch constant, so the merged
                # result IS the output tile; cp patches in the r-branch.
                o = io.tile([P, FMAX], F16, tag="o", name="o")[:, :w]
                nc.vector._custom_dve(
                    ops["gbw_pos" if pos else "gbw_neg"], out=o, in0=q1w,
                    in1=q2, s0=2 * w6 + bias, s1=10 * w6 + 2 * bias,
                    imm2=3 * w6 + bias)
                nc.vector.copy_predicated(o, mask, q1w)

                nc.scalar.dma_start(o_t[:, c0:c0 + w], o)

            # software pipeline, skew 1
            NP = len(pieces)
            for pi in range(NP + 1):
                if pi < NP:
                    stage_a(pi)
                if pi >= 1:
                    stage_b(pi - 1)

    nc.compile()
    return nc


def _get_nc(w6: float, bias: float, input_f32: bool):
    key = (w6, bias, tuple(PIECES), input_f32)
    if key not in _EXE_CACHE:
        _EXE_CACHE[key] = _build(w6, bias, input_f32)
    return _EXE_CACHE[key]


def _run(x, W, b, trace=False, tmpdir=None):
    from concourse.bass_utils import run_bass_kernel_spmd

    Wv = float(np.asarray(W).reshape(-1)[0])
    bv = float(np.asarray(b).reshape(-1)[0])
    w6 = Wv / 6.0

    nc = _get_nc(w6, bv, INPUT_F32)

    dt = np.float32 if INPUT_F32 else np.float16
    xs = np.asarray(x).astype(dt, copy=False)
    # [core, img, ch, P, col] -> [core, ch, P, img, col]: channel planes with
    # the core's 4 images contiguous along the free dim
    xs = np.ascontiguousarray(
        xs.reshape(N_CORES, IMGS_PER_CORE, 3, P, FREE).transpose(0, 2, 3, 1, 4)
    ).reshape(N_CORES, 3, P, FTOT)
    in_maps = [{"x": xs[i]} for i in range(N_CORES)]
    res = run_bass_kernel_spmd(
        nc, in_maps, list(range(N_CORES)), trace=trace, tmpdir=tmpdir
    )
    out = np.stack([np.asarray(res.results[i]["out"]) for i in range(N_CORES)],
                   axis=0)
    # [core, P, img*FREE] -> [core, img, P, FREE] -> [32,1,512,512]
    out = (out.astype(np.float32)
           .reshape(N_CORES, P, IMGS_PER_CORE, FREE)
           .transpose(0, 2, 1, 3)
           .reshape(32, 1, 512, 512))
    return out, res


def kernel(x, W, b):
    out, _ = _run(x, W, b, trace=False)
    return out


# revision 10
# speedup vs baseline: 1.0200x; 1.0200x over previous
"""RGB->hue + 1x1 conv (scalar scale+bias) Trainium2 Bass kernel.

Problem: x [32,3,512,512] f32 -> out [32,1,512,512] f32
  hue6 selected per argmax(r,g,b) branch (first-max, like torch):
    r max: (g-b)/delta  (mod 6)
    g max: (b-r)/delta + 2
    b max: (r-g)/delta + 4
  out = hue6 * (W/6) + b

Sharding: pure data parallel, 4 images per core on 8 cores. Host
re-lays x channel-major per core ([3, P, 8192], images along the free
dim) so compute pieces can be any size: small lead-in/lead-out pieces
hide DMA fill/drain, large middle pieces amortize instruction startup.

Formulation (divide-first, select-late). Let w6 = W/6, s = sign(w6).
With sign-folded diffs dgb' = s*(g-b), dbr' = s*(b-r) (operand swap at
trace time, zero cost):
  delta = max(|dgb'|,|dbr'|,|dgb'+dbr'|)   [custom DVE op ABSMAX3, via
          the sum-zero identity max3 = max(M3, -m3)]
  u  = |w6|/delta  (ACT Reciprocal, input scale 1/|w6|)
  q1w = dgb'*u + 6w6*[dgb'*u wraps]        [custom MULT_WRAP: the
        r-branch mod-6, folded into the multiply]
  q2 = dbr'*u
  m  = q1w>3w6 ? 10w6-(q1w+q2) : q2+2w6    [custom GB_WRAPPED: the g/b
        merge; 10w6 absorbs the +6w6 carried by wrapped q1w, so g/b
        outputs land in [w6,5w6] = w6*hue6 directly, no post-wrap]
  copy_predicated m <- q1w where r-max     [mask from ACT: r-max <=>
        s*(dbr + relu(dgb)) < 0, via max(dbr,dgb+dbr) = dbr+relu(dgb)]
  out = Copy(m) + bias (ACT, writes the fp16 output tile)

All intermediates fp16 (2x DVE modes; 2^-11 rel). Input is downcast to
fp16 on the host before sharding: halves input HBM traffic and makes
the first-touch subtracts 2x; delta can round to 0 for near-achromatic
pixels so the reciprocal gets a small eps bias (sim rel err 3.7e-3 vs
2e-2 budget). INPUT_F32 = True falls back to f32 input planes (exact
diffs, sim rel 2.7e-4) at 2x input DMA and 1x-subtract cost. Output is
fp16, upcast on host.
"""

import numpy as np

_EXE_CACHE: dict = {}

# Layout constants (x [32,3,512,512], 8 cores)
N_CORES = 8
IMGS_PER_CORE = 4
P = 128               # SBUF partitions
PLANE = 512 * 512     # elements per channel plane
FREE = PLANE // P     # 2048 free-dim elements per plane
FTOT = IMGS_PER_CORE * FREE  # 8192 free-dim elements per channel per core
# graded piece sizes: small ends for pipeline fill/drain, big middle
PIECES = [768, 1536, 2560, 2304, 1024]
assert sum(PIECES) == FTOT

INPUT_F32 = False     # False: host downcasts x to fp16 (half the input DMA)

_OPS_CACHE: dict = {}


def _register_custom_ops():
    """Register the fused DVE ops in concourse.dve_ops' runtime tables.
    Name->opcode rows are assigned past the stock OPS list; idempotent."""
    if _OPS_CACHE:
        return _OPS_CACHE
    import concourse.dve_ops as dops
    from concourse.dve_spec import (Spec, Src0, Src1, C0, C1, C2, Zero, maxx,
                                    minn, select, lower, _has_src1)
    from concourse.dve_uop import DveOpSpec

    existing = {op.name: op for op in dops.OPS}

    def make(name, spec):
        if name in existing:
            return existing[name]
        row = dops._CUSTOM_DVE_ROW_BASE + len(dops.OPS)
        shas = {}
        for ver in ("v3",):
            s = DveOpSpec(name=name, opcode=row, uops=lower(spec, ver=ver),
                          rd1_en=_has_src1(spec))
            shas[ver] = s.sha(ver)
        op = dops.DveOp(name, spec, subdim=False, uops_sha=shas)
        dops.OPS.append(op)
        dops.CUSTOM_DVE_SPECS[name] = spec
        dops._SUB_OPCODE_FOR_NAME[name] = row
        return op

    # delta = max(|a|,|b|,|a+b|): {a,b,-(a+b)} sums to zero, so the largest
    # magnitude is max(max3, -min3).
    t1 = Src0 + Src1
    c = Zero - t1
    M3 = maxx(maxx(Src0, Src1), c)
    m3 = minn(minn(Src0, Src1), c)
    spec_absmax3 = Spec(
        body=maxx(M3, Zero - m3),
        reference=lambda in0, in1, s0, s1, imm2: np.maximum(
            np.maximum(np.abs(in0.astype(np.float32)), np.abs(in1)),
            np.abs(in0.astype(np.float32) + in1)).astype(np.float32),
    )
    # q1w = p + C0*[p<0] + C1 (pos) / p + C0*[p>0] + C1 (neg), p = in0*in1,
    # C0 = 6w6, C1 = bias: the r-branch wrap AND the conv bias, fused into
    # the quotient multiply.
    p = Src0 * Src1
    spec_mw_pos = Spec(
        body=p + ((p < Zero) * C0) + C1,
        reference=lambda in0, in1, s0, s1, imm2: (
            in0.astype(np.float32) * in1
            + (in0.astype(np.float32) * in1 < 0) * s0 + s1).astype(np.float32),
    )
    spec_mw_neg = Spec(
        body=p + ((p > Zero) * C0) + C1,
        reference=lambda in0, in1, s0, s1, imm2: (
            in0.astype(np.float32) * in1
            + (in0.astype(np.float32) * in1 > 0) * s0 + s1).astype(np.float32),
    )
    # m = sel(b-branch, C1-(q1w+q2), q2+C0): C0=2w6, C1=10w6, C2(imm2)=3w6.
    # b-branch <=> q1w beyond 3w6 (wrapped q1w sits in [5w6,6w6]).
    tt = Src0 + Src1
    a = Src1 + C0
    b2 = C1 - tt
    spec_gbw_pos = Spec(
        body=select(Src0 > C2, b2, a),
        reference=lambda in0, in1, s0, s1, imm2: np.where(
            in0 > imm2, s1 - (in0.astype(np.float32) + in1),
            in1 + s0).astype(np.float32),
    )
    spec_gbw_neg = Spec(
        body=select(Src0 < C2, b2, a),
        reference=lambda in0, in1, s0, s1, imm2: np.where(
            in0 < imm2, s1 - (in0.astype(np.float32) + in1),
            in1 + s0).astype(np.float32),
    )
    _OPS_CACHE["absmax3"] = make("HUE_ABSMAX3_ANT", spec_absmax3)
    _OPS_CACHE["mw_pos"] = make("HUE_MW_POS_ANT", spec_mw_pos)
    _OPS_CACHE["mw_neg"] = make("HUE_MW_NEG_ANT", spec_mw_neg)
    _OPS_CACHE["gbw_pos"] = make("HUE_GBW_POS_ANT", spec_gbw_pos)
    _OPS_CACHE["gbw_neg"] = make("HUE_GBW_NEG_ANT", spec_gbw_neg)
    return _OPS_CACHE


def _build(w6: float, bias: float, input_f32: bool):
    """Trace the Bass kernel with W/6 and bias baked as immediates."""
    import concourse.bacc as bacc
    import concourse.tile as tile
    from concourse import mybir

    F16 = mybir.dt.float16
    F32 = mybir.dt.float32
    U8 = mybir.dt.uint8
    Alu = mybir.AluOpType
    Act = mybir.ActivationFunctionType

    ops = _register_custom_ops()
    pos = w6 >= 0
    aw6 = abs(w6)
    IN_DT = F32 if input_f32 else F16
    # fp16 input rounding can collapse delta to 0; keep 1/(delta/|w6|+eps)
    # finite there (q's are then 0 too, so the pixel lands on hue 2, bounded).
    recip_eps = 0.0 if input_f32 else 1e-4
    FMAX = max(PIECES)

    nc = bacc.Bacc("TRN2", target_bir_lowering=False, debug=False)

    def act_recip(out_ap, in_ap, scale, bias_f):
        ins = [
            nc.scalar.lower_ap(in_ap),
            mybir.ImmediateValue(dtype=F32, value=bias_f),
            mybir.ImmediateValue(dtype=F32, value=scale),
            mybir.ImmediateValue(dtype=F32, value=0.0),
        ]
        return nc.scalar.add_instruction(
            mybir.InstActivation(
                name=nc.get_next_instruction_name(),
                func=Act.Reciprocal,
                ins=ins,
                outs=[nc.scalar.lower_ap(out_ap)],
            )
        )

    x_t = nc.dram_tensor("x", [3, P, FTOT], IN_DT, kind="ExternalInput")
    o_t = nc.dram_tensor("out", [P, FTOT], F16, kind="ExternalOutput")

    with tile.TileContext(nc, pool_alloc_mode="queue") as tc:
        with (
            tc.tile_pool(name="io", bufs=3) as io,
            tc.tile_pool(name="tmp", bufs=2) as tmp,
        ):
            state = {}
            pieces = []
            c0 = 0
            for w in PIECES:
                pieces.append((c0, w))
                c0 += w

            def stage_a(pi):
                c0, w = pieces[pi]
                r = io.tile([P, FMAX], IN_DT, tag="r", name="r")[:, :w]
                g = io.tile([P, FMAX], IN_DT, tag="g", name="g")[:, :w]
                b = io.tile([P, FMAX], IN_DT, tag="b", name="b")[:, :w]
                nc.sync.dma_start(g, x_t[1, :, c0:c0 + w])
                nc.sync.dma_start(b, x_t[2, :, c0:c0 + w])
                nc.sync.dma_start(r, x_t[0, :, c0:c0 + w])

                # sign-folded diffs: dgb' = s*(g-b), dbr' = s*(b-r)
                dgb = tmp.tile([P, FMAX], F16, tag="dgb", name="dgb")[:, :w]
                dbr = tmp.tile([P, FMAX], F16, tag="dbr", name="dbr")[:, :w]
                if pos:
                    nc.vector.tensor_sub(dgb, g, b)
                    nc.vector.tensor_sub(dbr, b, r)
                else:
                    nc.vector.tensor_sub(dgb, b, g)
                    nc.vector.tensor_sub(dbr, r, b)

                delta = tmp.tile([P, FMAX], F16, tag="delta", name="delta")[:, :w]
                nc.vector._custom_dve(ops["absmax3"], out=delta, in0=dgb, in1=dbr)

                # r-max <=> s*(dbr + relu(dgb)) < 0; with primed operands:
                # pos: w = relu(dgb')+dbr' < 0 ; neg: w = dbr'-relu(-dgb') > 0
                re = tmp.tile([P, FMAX], F16, tag="re", name="re")[:, :w]
                nc.scalar.activation(re, dgb, Act.Relu,
                                     scale=1.0 if pos else -1.0)
                wv = tmp.tile([P, FMAX], F16, tag="wv", name="wv")[:, :w]
                if pos:
                    nc.vector.tensor_add(wv, re, dbr)
                else:
                    nc.vector.tensor_sub(wv, dbr, re)
                mask = tmp.tile([P, FMAX], U8, tag="mask", name="mask")[:, :w]
                nc.scalar.activation(mask, wv, Act.Relu,
                                     scale=-1e4 if pos else 1e4)

                u = tmp.tile([P, FMAX], F16, tag="u", name="u")[:, :w]
                act_recip(u, delta, scale=1.0 / aw6, bias_f=recip_eps)

                state[pi] = (dgb, dbr, u, mask)

            def stage_b(pi):
                c0, w = pieces[pi]
                dgb, dbr, u, mask = state.pop(pi)

                q1w = tmp.tile([P, FMAX], F16, tag="q1w", name="q1w")[:, :w]
                nc.vector._custom_dve(
                    ops["mw_pos" if pos else "mw_neg"], out=q1w, in0=dgb,
                    in1=u, s0=6 * w6, s1=bias)
                q2 = tmp.tile([P, FMAX], F16, tag="q2", name="q2")[:, :w]
                nc.vector.tensor_tensor(q2, dbr, u, op=Alu.mult)

                # bias is folded into every branch constant, so the merged
                # result IS the output tile; cp patches in the r-branch.
                o = io.tile([P, FMAX], F16, tag="o", name="o")[:, :w]
                nc.vector._custom_dve(
                    ops["gbw_pos" if pos else "gbw_neg"], out=o, in0=q1w,
                    in1=q2, s0=2 * w6 + bias, s1=10 * w6 + 2 * bias,
                    imm2=3 * w6 + bias)
                nc.vector.copy_predicated(o, mask, q1w)

                nc.scalar.dma_start(o_t[:, c0:c0 + w], o)

            # software pipeline, skew 1
            NP = len(pieces)
            for pi in range(NP + 1):
                if pi < NP:
                    stage_a(pi)
                if pi >= 1:
                    stage_b(pi - 1)

    nc.compile()
    return nc


def _get_nc(w6: float, bias: float, input_f32: bool):
    key = (w6, bias, tuple(PIECES), input_f32)
    if key not in _EXE_CACHE:
        _EXE_CACHE[key] = _build(w6, bias, input_f32)
    return _EXE_CACHE[key]


def _run(x, W, b, trace=False, tmpdir=None):
    from concourse.bass_utils import run_bass_kernel_spmd

    Wv = float(np.asarray(W).reshape(-1)[0])
    bv = float(np.asarray(b).reshape(-1)[0])
    w6 = Wv / 6.0

    nc = _get_nc(w6, bv, INPUT_F32)

    dt = np.float32 if INPUT_F32 else np.float16
    xs = np.asarray(x).astype(dt, copy=False)
    # [core, img, ch, P, col] -> [core, ch, P, img, col]: channel planes with
    # the core's 4 images contiguous along the free dim
    xs = np.ascontiguousarray(
        xs.reshape(N_CORES, IMGS_PER_CORE, 3, P, FREE).transpose(0, 2, 3, 1, 4)
    ).reshape(N_CORES, 3, P, FTOT)
    in_maps = [{"x": xs[i]} for i in range(N_CORES)]
    res = run_bass_kernel_spmd(
        nc, in_maps, list(range(N_CORES)), trace=trace, tmpdir=tmpdir
    )
    out = np.stack([np.asarray(res.results[i]["out"]) for i in range(N_CORES)],
                   axis=0)
    # [core, P, img*FREE] -> [core, img, P, FREE] -> [32,1,512,512]
    out = (out.astype(np.float32)
           .reshape(N_CORES, P, IMGS_PER_CORE, FREE)
           .transpose(0, 2, 1, 3)
           .reshape(32, 1, 512, 512))
    return out, res


def kernel(x, W, b):
    out, _ = _run(x, W, b, trace=False)
    return out


# revision 11
# speedup vs baseline: 1.0364x; 1.0161x over previous
"""RGB->hue + 1x1 conv (scalar scale+bias) Trainium2 Bass kernel.

Problem: x [32,3,512,512] f32 -> out [32,1,512,512] f32
  hue6 selected per argmax(r,g,b) branch (first-max, like torch):
    r max: (g-b)/delta  (mod 6)
    g max: (b-r)/delta + 2
    b max: (r-g)/delta + 4
  out = hue6 * (W/6) + b

Sharding: pure data parallel, 4 images per core on 8 cores. Host
re-lays x channel-major per core ([3, P, 8192], images along the free
dim) so compute pieces can be any size: small lead-in/lead-out pieces
hide DMA fill/drain, large middle pieces amortize instruction startup.

Formulation (divide-first, select-late). Let w6 = W/6, s = sign(w6).
With sign-folded diffs dgb' = s*(g-b), dbr' = s*(b-r) (operand swap at
trace time, zero cost):
  delta = max(|dgb'|,|dbr'|,|dgb'+dbr'|)   [custom DVE op ABSMAX3, via
          the sum-zero identity max3 = max(M3, -m3)]
  u  = |w6|/delta  (ACT Reciprocal, input scale 1/|w6|)
  q1w = dgb'*u + 6w6*[dgb'*u wraps]        [custom MULT_WRAP: the
        r-branch mod-6, folded into the multiply]
  q2 = dbr'*u
  m  = q1w>3w6 ? 10w6-(q1w+q2) : q2+2w6    [custom GB_WRAPPED: the g/b
        merge; 10w6 absorbs the +6w6 carried by wrapped q1w, so g/b
        outputs land in [w6,5w6] = w6*hue6 directly, no post-wrap]
  copy_predicated m <- q1w where r-max     [mask from ACT: r-max <=>
        s*(dbr + relu(dgb)) < 0, via max(dbr,dgb+dbr) = dbr+relu(dgb)]
  out = Copy(m) + bias (ACT, writes the fp16 output tile)

All intermediates fp16 (2x DVE modes; 2^-11 rel). Input is downcast to
fp16 on the host before sharding: halves input HBM traffic and makes
the first-touch subtracts 2x; delta can round to 0 for near-achromatic
pixels so the reciprocal gets a small eps bias (sim rel err 3.7e-3 vs
2e-2 budget). INPUT_F32 = True falls back to f32 input planes (exact
diffs, sim rel 2.7e-4) at 2x input DMA and 1x-subtract cost. Output is
fp16, upcast on host.
"""

import numpy as np

_EXE_CACHE: dict = {}

# Layout constants (x [32,3,512,512], 8 cores)
N_CORES = 8
IMGS_PER_CORE = 4
P = 128               # SBUF partitions
PLANE = 512 * 512     # elements per channel plane
FREE = PLANE // P     # 2048 free-dim elements per plane
FTOT = IMGS_PER_CORE * FREE  # 8192 free-dim elements per channel per core
# graded piece sizes: small ends for pipeline fill/drain, big middle
PIECES = [1024, 1792, 2560, 1792, 1024]
assert sum(PIECES) == FTOT

INPUT_F32 = False     # False: host downcasts x to fp16 (half the input DMA)

_OPS_CACHE: dict = {}


def _register_custom_ops():
    """Register the fused DVE ops in concourse.dve_ops' runtime tables.
    Name->opcode rows are assigned past the stock OPS list; idempotent."""
    if _OPS_CACHE:
        return _OPS_CACHE
    import concourse.dve_ops as dops
    from concourse.dve_spec import (Spec, Src0, Src1, C0, C1, C2, Zero, maxx,
                                    minn, select, lower, _has_src1)
    from concourse.dve_uop import DveOpSpec

    existing = {op.name: op for op in dops.OPS}

    def make(name, spec):
        if name in existing:
            return existing[name]
        row = dops._CUSTOM_DVE_ROW_BASE + len(dops.OPS)
        shas = {}
        for ver in ("v3",):
            s = DveOpSpec(name=name, opcode=row, uops=lower(spec, ver=ver),
                          rd1_en=_has_src1(spec))
            shas[ver] = s.sha(ver)
        op = dops.DveOp(name, spec, subdim=False, uops_sha=shas)
        dops.OPS.append(op)
        dops.CUSTOM_DVE_SPECS[name] = spec
        dops._SUB_OPCODE_FOR_NAME[name] = row
        return op

    # delta = max(|a|,|b|,|a+b|): {a,b,-(a+b)} sums to zero, so the largest
    # magnitude is max(max3, -min3).
    t1 = Src0 + Src1
    c = Zero - t1
    M3 = maxx(maxx(Src0, Src1), c)
    m3 = minn(minn(Src0, Src1), c)
    spec_absmax3 = Spec(
        body=maxx(M3, Zero - m3),
        reference=lambda in0, in1, s0, s1, imm2: np.maximum(
            np.maximum(np.abs(in0.astype(np.float32)), np.abs(in1)),
            np.abs(in0.astype(np.float32) + in1)).astype(np.float32),
    )
    # q1w = p + C0*[p<0] + C1 (pos) / p + C0*[p>0] + C1 (neg), p = in0*in1,
    # C0 = 6w6, C1 = bias: the r-branch wrap AND the conv bias, fused into
    # the quotient multiply.
    p = Src0 * Src1
    spec_mw_pos = Spec(
        body=p + ((p < Zero) * C0) + C1,
        reference=lambda in0, in1, s0, s1, imm2: (
            in0.astype(np.float32) * in1
            + (in0.astype(np.float32) * in1 < 0) * s0 + s1).astype(np.float32),
    )
    spec_mw_neg = Spec(
        body=p + ((p > Zero) * C0) + C1,
        reference=lambda in0, in1, s0, s1, imm2: (
            in0.astype(np.float32) * in1
            + (in0.astype(np.float32) * in1 > 0) * s0 + s1).astype(np.float32),
    )
    # m = sel(b-branch, C1-(q1w+q2), q2+C0): C0=2w6, C1=10w6, C2(imm2)=3w6.
    # b-branch <=> q1w beyond 3w6 (wrapped q1w sits in [5w6,6w6]).
    tt = Src0 + Src1
    a = Src1 + C0
    b2 = C1 - tt
    spec_gbw_pos = Spec(
        body=select(Src0 > C2, b2, a),
        reference=lambda in0, in1, s0, s1, imm2: np.where(
            in0 > imm2, s1 - (in0.astype(np.float32) + in1),
            in1 + s0).astype(np.float32),
    )
    spec_gbw_neg = Spec(
        body=select(Src0 < C2, b2, a),
        reference=lambda in0, in1, s0, s1, imm2: np.where(
            in0 < imm2, s1 - (in0.astype(np.float32) + in1),
            in1 + s0).astype(np.float32),
    )
    _OPS_CACHE["absmax3"] = make("HUE_ABSMAX3_ANT", spec_absmax3)
    _OPS_CACHE["mw_pos"] = make("HUE_MW_POS_ANT", spec_mw_pos)
    _OPS_CACHE["mw_neg"] = make("HUE_MW_NEG_ANT", spec_mw_neg)
    _OPS_CACHE["gbw_pos"] = make("HUE_GBW_POS_ANT", spec_gbw_pos)
    _OPS_CACHE["gbw_neg"] = make("HUE_GBW_NEG_ANT", spec_gbw_neg)
    return _OPS_CACHE


def _build(w6: float, bias: float, input_f32: bool):
    """Trace the Bass kernel with W/6 and bias baked as immediates."""
    import concourse.bacc as bacc
    import concourse.tile as tile
    from concourse import mybir

    F16 = mybir.dt.float16
    F32 = mybir.dt.float32
    U8 = mybir.dt.uint8
    Alu = mybir.AluOpType
    Act = mybir.ActivationFunctionType

    ops = _register_custom_ops()
    pos = w6 >= 0
    aw6 = abs(w6)
    IN_DT = F32 if input_f32 else F16
    # fp16 input rounding can collapse delta to 0; keep 1/(delta/|w6|+eps)
    # finite there (q's are then 0 too, so the pixel lands on hue 2, bounded).
    recip_eps = 0.0 if input_f32 else 1e-4
    FMAX = max(PIECES)

    nc = bacc.Bacc("TRN2", target_bir_lowering=False, debug=False)

    def act_recip(out_ap, in_ap, scale, bias_f):
        ins = [
            nc.scalar.lower_ap(in_ap),
            mybir.ImmediateValue(dtype=F32, value=bias_f),
            mybir.ImmediateValue(dtype=F32, value=scale),
            mybir.ImmediateValue(dtype=F32, value=0.0),
        ]
        return nc.scalar.add_instruction(
            mybir.InstActivation(
                name=nc.get_next_instruction_name(),
                func=Act.Reciprocal,
                ins=ins,
                outs=[nc.scalar.lower_ap(out_ap)],
            )
        )

    x_t = nc.dram_tensor("x", [3, P, FTOT], IN_DT, kind="ExternalInput")
    o_t = nc.dram_tensor("out", [P, FTOT], F16, kind="ExternalOutput")

    with tile.TileContext(nc, pool_alloc_mode="queue") as tc:
        with (
            tc.tile_pool(name="io", bufs=3) as io,
            tc.tile_pool(name="tmp", bufs=2) as tmp,
        ):
            state = {}
            pieces = []
            c0 = 0
            for w in PIECES:
                pieces.append((c0, w))
                c0 += w

            def stage_a(pi):
                c0, w = pieces[pi]
                r = io.tile([P, FMAX], IN_DT, tag="r", name="r")[:, :w]
                g = io.tile([P, FMAX], IN_DT, tag="g", name="g")[:, :w]
                b = io.tile([P, FMAX], IN_DT, tag="b", name="b")[:, :w]
                nc.sync.dma_start(g, x_t[1, :, c0:c0 + w])
                nc.sync.dma_start(b, x_t[2, :, c0:c0 + w])
                nc.sync.dma_start(r, x_t[0, :, c0:c0 + w])

                # sign-folded diffs: dgb' = s*(g-b), dbr' = s*(b-r)
                dgb = tmp.tile([P, FMAX], F16, tag="dgb", name="dgb")[:, :w]
                dbr = tmp.tile([P, FMAX], F16, tag="dbr", name="dbr")[:, :w]
                if pos:
                    nc.vector.tensor_sub(dgb, g, b)
                    nc.vector.tensor_sub(dbr, b, r)
                else:
                    nc.vector.tensor_sub(dgb, b, g)
                    nc.vector.tensor_sub(dbr, r, b)

                delta = tmp.tile([P, FMAX], F16, tag="delta", name="delta")[:, :w]
                nc.vector._custom_dve(ops["absmax3"], out=delta, in0=dgb, in1=dbr)

                # r-max <=> s*(dbr + relu(dgb)) < 0; with primed operands:
                # pos: w = relu(dgb')+dbr' < 0 ; neg: w = dbr'-relu(-dgb') > 0
                re = tmp.tile([P, FMAX], F16, tag="re", name="re")[:, :w]
                nc.scalar.activation(re, dgb, Act.Relu,
                                     scale=1.0 if pos else -1.0)
                wv = tmp.tile([P, FMAX], F16, tag="wv", name="wv")[:, :w]
                if pos:
                    nc.vector.tensor_add(wv, re, dbr)
                else:
                    nc.vector.tensor_sub(wv, dbr, re)
                mask = tmp.tile([P, FMAX], U8, tag="mask", name="mask")[:, :w]
                nc.scalar.activation(mask, wv, Act.Relu,
                                     scale=-1e4 if pos else 1e4)

                u = tmp.tile([P, FMAX], F16, tag="u", name="u")[:, :w]
                act_recip(u, delta, scale=1.0 / aw6, bias_f=recip_eps)

                state[pi] = (dgb, dbr, u, mask)

            def stage_b(pi):
                c0, w = pieces[pi]
                dgb, dbr, u, mask = state.pop(pi)

                q1w = tmp.tile([P, FMAX], F16, tag="q1w", name="q1w")[:, :w]
                nc.vector._custom_dve(
                    ops["mw_pos" if pos else "mw_neg"], out=q1w, in0=dgb,
                    in1=u, s0=6 * w6, s1=bias)
                q2 = tmp.tile([P, FMAX], F16, tag="q2", name="q2")[:, :w]
                nc.vector.tensor_tensor(q2, dbr, u, op=Alu.mult)

                # bias is folded into every branch constant, so the merged
                # result IS the output tile; cp patches in the r-branch.
                o = io.tile([P, FMAX], F16, tag="o", name="o")[:, :w]
                nc.vector._custom_dve(
                    ops["gbw_pos" if pos else "gbw_neg"], out=o, in0=q1w,
                    in1=q2, s0=2 * w6 + bias, s1=10 * w6 + 2 * bias,
                    imm2=3 * w6 + bias)
                nc.vector.copy_predicated(o, mask, q1w)

                nc.scalar.dma_start(o_t[:, c0:c0 + w], o)

            # software pipeline, skew 1
            NP = len(pieces)
            for pi in range(NP + 1):
                if pi < NP:
                    stage_a(pi)
                if pi >= 1:
                    stage_b(pi - 1)

    nc.compile()
    return nc


def _get_nc(w6: float, bias: float, input_f32: bool):
    key = (w6, bias, tuple(PIECES), input_f32)
    if key not in _EXE_CACHE:
        _EXE_CACHE[key] = _build(w6, bias, input_f32)
    return _EXE_CACHE[key]


def _run(x, W, b, trace=False, tmpdir=None):
    from concourse.bass_utils import run_bass_kernel_spmd

    Wv = float(np.asarray(W).reshape(-1)[0])
    bv = float(np.asarray(b).reshape(-1)[0])
    w6 = Wv / 6.0

    nc = _get_nc(w6, bv, INPUT_F32)

    dt = np.float32 if INPUT_F32 else np.float16
    xs = np.asarray(x).astype(dt, copy=False)
    # [core, img, ch, P, col] -> [core, ch, P, img, col]: channel planes with
    # the core's 4 images contiguous along the free dim
    xs = np.ascontiguousarray(
        xs.reshape(N_CORES, IMGS_PER_CORE, 3, P, FREE).transpose(0, 2, 3, 1, 4)
    ).reshape(N_CORES, 3, P, FTOT)
    in_maps = [{"x": xs[i]} for i in range(N_CORES)]
    res = run_bass_kernel_spmd(
        nc, in_maps, list(range(N_CORES)), trace=trace, tmpdir=tmpdir
    )
    out = np.stack([np.asarray(res.results[i]["out"]) for i in range(N_CORES)],
                   axis=0)
    # [core, P, img*FREE] -> [core, img, P, FREE] -> [32,1,512,512]
    out = (out.astype(np.float32)
           .reshape(N_CORES, P, IMGS_PER_CORE, FREE)
           .transpose(0, 2, 1, 3)
           .reshape(32, 1, 512, 512))
    return out, res


def kernel(x, W, b):
    out, _ = _run(x, W, b, trace=False)
    return out
